# revision 17
# baseline (speedup 1.0000x reference)
"""DifColorQuantization Trainium2 kernel.

Math (per pixel p, codebook color k):
    ref:  argmin_k sqrt(sum_c (x_c - cb_kc + eps)^2 + eps) ; out = cb[argmin]
    sqrt/+eps are monotone, so rank by the k-dependent part of the expanded
    square:  s_k = sum_c w_kc * x_c + b_k,  w_kc = 2*(eps-cb_kc),
    b_k = sum_c (eps-cb_kc)^2  (the sum_c x_c^2 term is k-independent).

Device pipeline per core (H sharded 8 ways, 131072 px/core, 32 supertiles
of 4096 px = 1024 cols x 4 q-slots; block b = 128 pixel columns):
    1. PE scores, transposed layout: per block, ONE bf16 matmul with
       lhsT = image chunk [75, 128] (stationary), rhs = stacked weights
       [75, 128] -> PSUM [128 px, (q,k)].  The 75 contract rows hold the
       fp32 x fp32 products expanded into bf16-term pairs (x split into 3
       bf16 terms, w into 3; all pairs with i+j<=2 stacked along the
       contract dim, + 3 ones rows for the 3 bias terms).  PE matmul cost
       depends only on the 128 moving columns, so this runs 4x faster
       than an fp32 matmul at ~fp32 accuracy (score err ~1e-7).
    2. DVE reduce min over k segments -> m [128, 32]
    3. DVE tensor_tensor is_equal(scores_T, m broadcast via a zero-stride
       AP) -> one-hot_T bf16 in SBUF.  (Pool/GPSIMD cannot run
       TensorTensor nor read PSUM on this hardware, so DVE owns both the
       min and the compare; DVE is the bottleneck engine.)
    4. PE transpose per 128-block -> one-hot [(q,k), px] in PSUM; one ACT
       evict -> SBUF.  (XBAR DMA transposes measured ~10us/supertile on
       real hw vs the 0.9us sim model - do not use them here.)
    5. PE gather: 2 matmuls, lhsT = block-diag bf16 codebook [128, 32]
       (12 live cols padded to 32), halves writing PSUM partitions
       {0..31} / {32..63}; ACT evict (bf16) -> SBUF; DMA out 1KB per
       partition over 64 partitions.

Numerics: scores match the fp32 affine form to ~1e-7, so only pixels
with near-tied top-2 distances can flip to the other color (a handful
of 1M); the gather codebook is rounded to bf16 (rel err ~1e-3 vs the
2e-2 budget) and the output is returned as bf16 codebook colors
(lossless for single-hot pixels).  Measured rel-l2 error vs the fp32
reference on hardware: 2.55e-3.
"""

import numpy as np

H = 1024
W = 1024
K = 32
EPS = 1e-6
NCORES = 8
ROWS = H // NCORES            # 128 rows per core
NPX = ROWS * W                # 131072 pixels per core
TILE_PX = 2048                # pixels per tile (4 slots x 512)
NSLOT = 4
SLOT_N = 512                  # columns per slot
NT = NPX // TILE_PX           # 64 tiles

# bf16 term pairs (x_term_i, w_term_j) stacked along the matmul contract
# dim; i+j<=2 keeps every product term down to ~2^-26 relative.
PAIRS = [(0, 0), (0, 1), (1, 0), (1, 1), (0, 2), (2, 0)]
NBIAS = 3                     # bias bf16 terms (ones rows)
NSTK = 12 * len(PAIRS) + NBIAS  # 75 contract rows


def _build_program(n_tiles, reps=1):
    import concourse.bass as bass
    import concourse.bacc as bacc
    import concourse.tile as tile
    from concourse import mybir

    f32 = mybir.dt.float32
    bf16 = mybir.dt.bfloat16

    nc = bacc.Bacc(None, target_bir_lowering=False)
    # x rows: the stacked bf16 image terms (see PAIRS) + 3 ones rows.
    # col 512t+n <-> pixel 2048t + 512q + n (4 pixels per column, one per
    # q slot, channel rows 4c+q within each 12-row pair block).
    L = SLOT_N * n_tiles
    x = nc.dram_tensor("x", [NSTK, L], bf16, kind="ExternalInput")
    wstk = nc.dram_tensor("wstk", [NSTK, 128], bf16, kind="ExternalInput")
    # cbt cols [0:32]: gather codebook, block-diag (padded to 32 output
    # partitions so the whole evict region is written):
    # cbt[32q+k, 4c+q] = cb[k, c].  cols [32:160]: bf16 identity.
    cbt = nc.dram_tensor("cbt", [128, 160], bf16, kind="ExternalInput")
    # y[32h + 4c+q, 512s + n] = channel c of pixel col 1024s + 512h + n
    # (bf16 is lossless here: single-hot colors are exactly the bf16
    # codebook entries)
    y = nc.dram_tensor("y", [64, L // 2], bf16, kind="ExternalOutput")

    assert n_tiles % 2 == 0
    n_super = n_tiles // 2
    SUP = 2 * SLOT_N  # 1024 cols per supertile, 2 PSUM banks
    QCOL = SUP // 2   # 512 cols per gather half
    CSPL = 320        # colors-evict column split between ACT and DVE
    with tile.TileContext(nc) as tc:
        with (
            tc.tile_pool(name="const", bufs=1) as constp,
            tc.tile_pool(name="io", bufs=1) as iop,
            tc.tile_pool(name="work", bufs=3) as workp,
            tc.tile_pool(name="ps", bufs=2, space=bass.MemorySpace.PSUM) as psp,
            tc.tile_pool(name="pso", bufs=2, space=bass.MemorySpace.PSUM) as psop,
            tc.tile_pool(name="psq", bufs=2, space=bass.MemorySpace.PSUM) as psq,
        ):
            cbt_t = constp.tile([128, 160], bf16)
            nc.sync.dma_start(cbt_t[:], cbt[:])
            cb_t = cbt_t[:, 0:32]
            iden_t = cbt_t[:, 32:160]
            w_t = constp.tile([NSTK, 128], bf16)
            nc.sync.dma_start(w_t[:], wstk[:])

            img = iop.tile([NSTK, L], bf16, tag="img")
            nc.sync.dma_start(img[:], x[:])

            def _super(s):
                # transposed scores: 8 blocks of [128 px, (q,k)]
                ps_T = psp.tile([128, SUP], f32, tag="ps_T")
                for b in range(8):
                    col = SUP * s + 128 * b
                    nc.tensor.matmul(
                        ps_T[:, 128 * b : 128 * (b + 1)],
                        img[:, col : col + 128],
                        w_t,
                    )

                # per-pixel min over the 32 scores (DVE, straight from PSUM)
                m = workp.tile([128, 32], f32, tag="m")
                nc.vector.tensor_reduce(
                    m[:],
                    ps_T[:].rearrange("p (s k) -> p s k", k=K),
                    axis=mybir.AxisListType.X,
                    op=mybir.AluOpType.min,
                )

                # one-hot in transposed layout (DVE; m broadcast along k
                # via a zero-stride AP).  GPSIMD cannot run TensorTensor or
                # touch PSUM on this hardware, so DVE owns both passes.
                onehot = workp.tile([128, SUP], bf16, tag="onehot")
                nc.vector.tensor_tensor(
                    onehot[:].rearrange("p (s k) -> p s k", k=K),
                    ps_T[:].rearrange("p (s k) -> p s k", k=K),
                    m[:].to_broadcast((128, 32, K)),
                    op=mybir.AluOpType.is_equal,
                )

                # transpose back to [(q,k), px] per block on the PE, then
                # one ACT evict (XBAR DMA transpose measured ~5x slower on
                # real hw than the sim model)
                ps_O = psop.tile([128, SUP], bf16, tag="ps_O")
                for b in range(8):
                    nc.tensor.transpose(
                        ps_O[:, 128 * b : 128 * (b + 1)],
                        onehot[:, 128 * b : 128 * (b + 1)],
                        iden_t,
                    )
                oh_sb = workp.tile([128, SUP], bf16, tag="oh_sb")
                nc.scalar.activation(
                    oh_sb[:], ps_O[:], mybir.ActivationFunctionType.Copy
                )

                # gather colors: half h -> PSUM partitions 32h..32h+11
                ps_o = psq.tile([64, QCOL], f32, tag="ps_o")
                for g in range(2):
                    nc.tensor.matmul(
                        ps_o[32 * g : 32 * g + 32, :],
                        cb_t,
                        oh_sb[:, QCOL * g : QCOL * (g + 1)],
                    )
                # colors evict on ACT (DVE is the bottleneck engine)
                o_sb = workp.tile([64, QCOL], bf16, tag="o_sb")
                nc.scalar.activation(
                    o_sb[:], ps_o[:], mybir.ActivationFunctionType.Copy
                )
                nc.sync.dma_start(y[:, QCOL * s : QCOL * (s + 1)], o_sb[:])

            def _body():
                for s in range(n_super):
                    _super(s)

            if reps == 1:
                _body()
            else:
                # hardware loop: used only for timing (program size stays
                # constant while the iteration count varies)
                with tc.For_i(0, reps, 1):
                    _body()
    nc.compile()
    return nc


def _bf16_terms(a, n):
    """Split a float array into n bf16 terms (greedy residual)."""
    import ml_dtypes

    terms = []
    r = np.asarray(a, np.float64)
    for _ in range(n):
        t = r.astype(np.float32).astype(ml_dtypes.bfloat16)
        terms.append(t)
        r = r - t.astype(np.float64)
    return terms


def _host_consts(printability_array):
    """Pack kernel constants: wstk [75,128] bf16 and cbt [128,12] bf16."""
    import ml_dtypes

    cb = np.asarray(printability_array, np.float64).reshape(K, 3)
    w = 2.0 * (EPS - cb)                       # [K, 3]
    b = np.sum((EPS - cb) ** 2, axis=1)        # [K]
    wt = _bf16_terms(w, 3)
    bt = _bf16_terms(b, NBIAS)

    wstk = np.zeros((NSTK, 128), ml_dtypes.bfloat16)
    for p, (_, j) in enumerate(PAIRS):
        for q in range(NSLOT):
            for c in range(3):
                for k in range(K):
                    wstk[12 * p + 4 * c + q, 32 * q + k] = wt[j][k, c]
    for j in range(NBIAS):
        for q in range(NSLOT):
            wstk[12 * len(PAIRS) + j, 32 * q : 32 * q + K] = bt[j]

    cbt = np.zeros((128, 160), ml_dtypes.bfloat16)
    cbf = cb.astype(np.float32).astype(ml_dtypes.bfloat16)
    for q in range(NSLOT):
        for k in range(K):
            for c in range(3):
                cbt[32 * q + k, 4 * c + q] = cbf[k, c]
    cbt[:, 32:160] = np.eye(128, dtype=ml_dtypes.bfloat16)
    return wstk, cbt


_PROG_CACHE = {}


def _pack_x(flat3):
    """[3, npx] fp32 -> [75, npx/4] bf16 stacked-term image."""
    import ml_dtypes

    npx = flat3.shape[1]
    nt = npx // TILE_PX
    ncols = nt * SLOT_N
    v = flat3.reshape(3, nt, NSLOT, SLOT_N)          # (c, t, q, n)
    v12 = v.transpose(0, 2, 1, 3).reshape(12, ncols)  # rows 4c+q
    xt = _bf16_terms(v12, 3)
    out = np.empty((NSTK, ncols), ml_dtypes.bfloat16)
    for p, (i, _) in enumerate(PAIRS):
        out[12 * p : 12 * (p + 1)] = xt[i]
    out[12 * len(PAIRS) :] = 1.0
    return out


def _unpack_y(y128):
    """[64, ncols/2] -> [3, npx] inverse of the device output layout."""
    ncols2 = y128.shape[1]
    n_super = ncols2 // 512
    # y128[32h + j, 512s + n] = v12[j, 1024s + 512h + n], j = 4c+q
    yr = np.asarray(y128).astype(np.float32).reshape(2, 32, n_super, 512)[:, :12]
    v12 = yr.transpose(1, 2, 0, 3).reshape(12, n_super * 1024)
    nt = (ncols2 * 2) // SLOT_N
    v = v12.reshape(3, NSLOT, nt, SLOT_N)            # (c, q, t, n)
    return v.transpose(0, 2, 1, 3).reshape(3, nt * TILE_PX)


def kernel(adv_patch, printability_array):
    from concourse.bass_utils import run_bass_kernel_spmd

    adv_patch = np.ascontiguousarray(adv_patch, dtype=np.float32)
    wstk, cbt = _host_consts(printability_array)

    if NT not in _PROG_CACHE:
        _PROG_CACHE[NT] = _build_program(NT)
    nc = _PROG_CACHE[NT]

    in_maps = []
    for i in range(NCORES):
        xs = adv_patch[:, i * ROWS : (i + 1) * ROWS, :].reshape(3, NPX)
        in_maps.append({"x": _pack_x(xs), "wstk": wstk, "cbt": cbt})

    res = run_bass_kernel_spmd(nc, in_maps, list(range(NCORES)))

    out = np.empty((1, 3, H, W), np.float32)
    for i in range(NCORES):
        out[0, :, i * ROWS : (i + 1) * ROWS, :] = _unpack_y(
            res.results[i]["y"]
        ).reshape(3, ROWS, W)
    return out


# revision 20
# speedup vs baseline: 1.0921x; 1.0921x over previous
"""DifColorQuantization Trainium2 kernel.

Math (per pixel p, codebook color k):
    ref:  argmin_k sqrt(sum_c (x_c - cb_kc + eps)^2 + eps) ; out = cb[argmin]
    sqrt/+eps are monotone, so rank by the k-dependent part of the expanded
    square:  s_k = sum_c w_kc * x_c + b_k,  w_kc = 2*(eps-cb_kc),
    b_k = sum_c (eps-cb_kc)^2  (the sum_c x_c^2 term is k-independent).

Device pipeline per core (H sharded 8 ways, 131072 px/core, 32 supertiles
of 4096 px = 1024 cols x 4 q-slots; block b = 128 pixel columns):
    1. PE scores, transposed layout: per block, ONE bf16 matmul with
       lhsT = image chunk [75, 128] (stationary), rhs = stacked weights
       [75, 128] -> PSUM [128 px, (q,k)].  The 75 contract rows hold the
       fp32 x fp32 products expanded into bf16-term pairs (x split into 3
       bf16 terms, w into 3; all pairs with i+j<=2 stacked along the
       contract dim, + 3 ones rows for the 3 bias terms).  PE matmul cost
       depends only on the 128 moving columns, so this runs 4x faster
       than an fp32 matmul at ~fp32 accuracy (score err ~1e-7).
    2. DVE reduce min over k segments -> m [128, 32]
    3. DVE tensor_tensor is_equal(scores_T, m broadcast via a zero-stride
       AP) -> one-hot_T bf16 in SBUF.  (Pool/GPSIMD cannot run
       TensorTensor nor read PSUM on this hardware, so DVE owns both the
       min and the compare; DVE is the bottleneck engine.)
    4. PE transpose per 128-block -> one-hot [(q,k), px] in PSUM; one ACT
       evict -> SBUF.  (XBAR DMA transposes measured ~10us/supertile on
       real hw vs the 0.9us sim model - do not use them here.)
    5. PE gather: 2 matmuls, lhsT = block-diag bf16 codebook [128, 32]
       (12 live cols padded to 32), halves writing PSUM partitions
       {0..31} / {32..63}; ACT evict (bf16) -> SBUF; DMA out 1KB per
       partition over 64 partitions.

Numerics: scores match the fp32 affine form to ~1e-7, so only pixels
with near-tied top-2 distances can flip to the other color (a handful
of 1M); the gather codebook is rounded to bf16 (rel err ~1e-3 vs the
2e-2 budget) and the output is returned as bf16 codebook colors
(lossless for single-hot pixels).  Measured rel-l2 error vs the fp32
reference on hardware: 2.55e-3.
"""

import numpy as np

H = 1024
W = 1024
K = 32
EPS = 1e-6
NCORES = 8
ROWS = H // NCORES            # 128 rows per core
NPX = ROWS * W                # 131072 pixels per core
TILE_PX = 2048                # pixels per tile (4 slots x 512)
NSLOT = 4
SLOT_N = 512                  # columns per slot
NT = NPX // TILE_PX           # 64 tiles

# bf16 term pairs (x_term_i, w_term_j) stacked along the matmul contract
# dim; i+j<=2 keeps every product term down to ~2^-26 relative.
PAIRS = [(0, 0), (0, 1), (1, 0), (1, 1), (0, 2), (2, 0)]
NBIAS = 3                     # bias bf16 terms (ones rows)
NSTK = 12 * len(PAIRS) + NBIAS  # 75 contract rows


def _build_program(n_tiles, reps=1):
    import concourse.bass as bass
    import concourse.bacc as bacc
    import concourse.tile as tile
    from concourse import mybir

    f32 = mybir.dt.float32
    bf16 = mybir.dt.bfloat16

    nc = bacc.Bacc(None, target_bir_lowering=False)
    # x rows: the stacked bf16 image terms (see PAIRS) + 3 ones rows.
    # col 512t+n <-> pixel 2048t + 512q + n (4 pixels per column, one per
    # q slot, channel rows 4c+q within each 12-row pair block).
    L = SLOT_N * n_tiles
    x = nc.dram_tensor("x", [NSTK, L], bf16, kind="ExternalInput")
    wstk = nc.dram_tensor("wstk", [NSTK, 128], bf16, kind="ExternalInput")
    # cbt cols [0:32]: gather codebook, block-diag (padded to 32 output
    # partitions so the whole evict region is written):
    # cbt[32q+k, 4c+q] = cb[k, c].  cols [32:160]: bf16 identity.
    cbt = nc.dram_tensor("cbt", [128, 160], bf16, kind="ExternalInput")
    # y[32h + 4c+q, 512s + n] = channel c of pixel col 1024s + 512h + n
    # (bf16 is lossless here: single-hot colors are exactly the bf16
    # codebook entries)
    y = nc.dram_tensor("y", [64, L // 2], bf16, kind="ExternalOutput")

    assert n_tiles % 2 == 0
    n_super = n_tiles // 2
    SUP = 2 * SLOT_N  # 1024 cols per supertile, 2 PSUM banks
    QCOL = SUP // 2   # 512 cols per gather half
    with tile.TileContext(nc) as tc:
        with (
            tc.tile_pool(name="const", bufs=1) as constp,
            tc.tile_pool(name="io", bufs=1) as iop,
            tc.tile_pool(name="work", bufs=4) as workp,
            tc.tile_pool(name="ps", bufs=3, space=bass.MemorySpace.PSUM) as psp,
            tc.tile_pool(name="pso", bufs=1, space=bass.MemorySpace.PSUM) as psop,
            tc.tile_pool(name="psq", bufs=1, space=bass.MemorySpace.PSUM) as psq,
        ):
            cbt_t = constp.tile([128, 160], bf16)
            nc.sync.dma_start(cbt_t[:], cbt[:])
            cb_t = cbt_t[:, 0:32]
            iden_t = cbt_t[:, 32:160]
            w_t = constp.tile([NSTK, 128], bf16)
            nc.sync.dma_start(w_t[:], wstk[:])

            img = iop.tile([NSTK, L], bf16, tag="img")
            nc.sync.dma_start(img[:], x[:])

            def _super(s):
                # transposed scores: 8 blocks of [128 px, (q,k)]
                ps_T = psp.tile([128, SUP], f32, tag="ps_T")
                for b in range(8):
                    col = SUP * s + 128 * b
                    nc.tensor.matmul(
                        ps_T[:, 128 * b : 128 * (b + 1)],
                        img[:, col : col + 128],
                        w_t,
                    )

                # per-pixel min over the 32 scores (DVE, straight from PSUM)
                m = workp.tile([128, 32], f32, tag="m")
                nc.vector.tensor_reduce(
                    m[:],
                    ps_T[:].rearrange("p (s k) -> p s k", k=K),
                    axis=mybir.AxisListType.X,
                    op=mybir.AluOpType.min,
                )

                # one-hot in transposed layout (DVE; m broadcast along k
                # via a zero-stride AP).  GPSIMD cannot run TensorTensor or
                # touch PSUM on this hardware, so DVE owns both passes.
                onehot = workp.tile([128, SUP], bf16, tag="onehot")
                nc.vector.tensor_tensor(
                    onehot[:].rearrange("p (s k) -> p s k", k=K),
                    ps_T[:].rearrange("p (s k) -> p s k", k=K),
                    m[:].to_broadcast((128, 32, K)),
                    op=mybir.AluOpType.is_equal,
                )

                # transpose back to [(q,k), px] per block on the PE, then
                # one ACT evict (XBAR DMA transpose measured ~5x slower on
                # real hw than the sim model)
                ps_O = psop.tile([128, SUP], bf16, tag="ps_O")
                for b in range(8):
                    nc.tensor.transpose(
                        ps_O[:, 128 * b : 128 * (b + 1)],
                        onehot[:, 128 * b : 128 * (b + 1)],
                        iden_t,
                    )
                oh_sb = workp.tile([128, SUP], bf16, tag="oh_sb")
                nc.scalar.activation(
                    oh_sb[:], ps_O[:], mybir.ActivationFunctionType.Copy
                )

                # gather colors: half h -> PSUM partitions 32h..32h+11
                ps_o = psq.tile([64, QCOL], f32, tag="ps_o")
                for g in range(2):
                    nc.tensor.matmul(
                        ps_o[32 * g : 32 * g + 32, :],
                        cb_t,
                        oh_sb[:, QCOL * g : QCOL * (g + 1)],
                    )
                # colors evict on ACT (DVE is the bottleneck engine)
                o_sb = workp.tile([64, QCOL], bf16, tag="o_sb")
                nc.scalar.activation(
                    o_sb[:], ps_o[:], mybir.ActivationFunctionType.Copy
                )
                nc.sync.dma_start(y[:, QCOL * s : QCOL * (s + 1)], o_sb[:])

            def _body():
                for s in range(n_super):
                    _super(s)

            if reps == 1:
                _body()
            else:
                # hardware loop: used only for timing (program size stays
                # constant while the iteration count varies).  Two bodies
                # per For_i iteration amortize loop-control overhead.
                assert reps % 2 == 0
                with tc.For_i(0, reps // 2, 1):
                    _body()
                    _body()
    nc.compile()
    return nc


def _bf16_terms(a, n):
    """Split a float array into n bf16 terms (greedy residual)."""
    import ml_dtypes

    terms = []
    r = np.asarray(a, np.float64)
    for _ in range(n):
        t = r.astype(np.float32).astype(ml_dtypes.bfloat16)
        terms.append(t)
        r = r - t.astype(np.float64)
    return terms


def _host_consts(printability_array):
    """Pack kernel constants: wstk [75,128] bf16 and cbt [128,12] bf16."""
    import ml_dtypes

    cb = np.asarray(printability_array, np.float64).reshape(K, 3)
    w = 2.0 * (EPS - cb)                       # [K, 3]
    b = np.sum((EPS - cb) ** 2, axis=1)        # [K]
    wt = _bf16_terms(w, 3)
    bt = _bf16_terms(b, NBIAS)

    wstk = np.zeros((NSTK, 128), ml_dtypes.bfloat16)
    for p, (_, j) in enumerate(PAIRS):
        for q in range(NSLOT):
            for c in range(3):
                for k in range(K):
                    wstk[12 * p + 4 * c + q, 32 * q + k] = wt[j][k, c]
    for j in range(NBIAS):
        for q in range(NSLOT):
            wstk[12 * len(PAIRS) + j, 32 * q : 32 * q + K] = bt[j]

    cbt = np.zeros((128, 160), ml_dtypes.bfloat16)
    cbf = cb.astype(np.float32).astype(ml_dtypes.bfloat16)
    for q in range(NSLOT):
        for k in range(K):
            for c in range(3):
                cbt[32 * q + k, 4 * c + q] = cbf[k, c]
    cbt[:, 32:160] = np.eye(128, dtype=ml_dtypes.bfloat16)
    return wstk, cbt


_PROG_CACHE = {}


def _pack_x(flat3):
    """[3, npx] fp32 -> [75, npx/4] bf16 stacked-term image."""
    import ml_dtypes

    npx = flat3.shape[1]
    nt = npx // TILE_PX
    ncols = nt * SLOT_N
    v = flat3.reshape(3, nt, NSLOT, SLOT_N)          # (c, t, q, n)
    v12 = v.transpose(0, 2, 1, 3).reshape(12, ncols)  # rows 4c+q
    xt = _bf16_terms(v12, 3)
    out = np.empty((NSTK, ncols), ml_dtypes.bfloat16)
    for p, (i, _) in enumerate(PAIRS):
        out[12 * p : 12 * (p + 1)] = xt[i]
    out[12 * len(PAIRS) :] = 1.0
    return out


def _unpack_y(y128):
    """[64, ncols/2] -> [3, npx] inverse of the device output layout."""
    ncols2 = y128.shape[1]
    n_super = ncols2 // 512
    # y128[32h + j, 512s + n] = v12[j, 1024s + 512h + n], j = 4c+q
    yr = np.asarray(y128).astype(np.float32).reshape(2, 32, n_super, 512)[:, :12]
    v12 = yr.transpose(1, 2, 0, 3).reshape(12, n_super * 1024)
    nt = (ncols2 * 2) // SLOT_N
    v = v12.reshape(3, NSLOT, nt, SLOT_N)            # (c, q, t, n)
    return v.transpose(0, 2, 1, 3).reshape(3, nt * TILE_PX)


def kernel(adv_patch, printability_array):
    from concourse.bass_utils import run_bass_kernel_spmd

    adv_patch = np.ascontiguousarray(adv_patch, dtype=np.float32)
    wstk, cbt = _host_consts(printability_array)

    if NT not in _PROG_CACHE:
        _PROG_CACHE[NT] = _build_program(NT)
    nc = _PROG_CACHE[NT]

    in_maps = []
    for i in range(NCORES):
        xs = adv_patch[:, i * ROWS : (i + 1) * ROWS, :].reshape(3, NPX)
        in_maps.append({"x": _pack_x(xs), "wstk": wstk, "cbt": cbt})

    res = run_bass_kernel_spmd(nc, in_maps, list(range(NCORES)))

    out = np.empty((1, 3, H, W), np.float32)
    for i in range(NCORES):
        out[0, :, i * ROWS : (i + 1) * ROWS, :] = _unpack_y(
            res.results[i]["y"]
        ).reshape(3, ROWS, W)
    return out


# revision 21
# speedup vs baseline: 1.3906x; 1.2733x over previous
"""DifColorQuantization Trainium2 kernel.

Math (per pixel p, codebook color k):
    ref:  argmin_k sqrt(sum_c (x_c - cb_kc + eps)^2 + eps) ; out = cb[argmin]
    sqrt/+eps are monotone, so rank by the k-dependent part of the expanded
    square:  s_k = sum_c w_kc * x_c + b_k,  w_kc = 2*(eps-cb_kc),
    b_k = sum_c (eps-cb_kc)^2  (the sum_c x_c^2 term is k-independent).

Device pipeline per core (H sharded 8 ways, 131072 px/core, 32 supertiles
of 4096 px = 1024 cols x 4 q-slots; block b = 128 pixel columns):
    1. PE scores, transposed layout: per block, ONE bf16 matmul with
       lhsT = image chunk [75, 128] (stationary), rhs = stacked weights
       [75, 128] -> PSUM [128 px, (q,k)].  The 75 contract rows hold the
       fp32 x fp32 products expanded into bf16-term pairs (x split into 3
       bf16 terms, w into 3; all pairs with i+j<=2 stacked along the
       contract dim, + 3 ones rows for the 3 bias terms).  PE matmul cost
       depends only on the 128 moving columns, so this runs 4x faster
       than an fp32 matmul at ~fp32 accuracy (score err ~1e-7).
    2. DVE reduce min over k segments -> m [128, 32]
    3. DVE tensor_tensor is_equal(scores_T, m broadcast via a zero-stride
       AP) -> one-hot_T bf16 in SBUF.  (Pool/GPSIMD cannot run
       TensorTensor nor read PSUM on this hardware, so DVE owns both the
       min and the compare; DVE is the bottleneck engine.)
    4. PE transpose per 128-block -> one-hot [(q,k), px] in PSUM; one ACT
       evict -> SBUF.  (XBAR DMA transposes measured ~10us/supertile on
       real hw vs the 0.9us sim model - do not use them here.)
    5. PE gather: 2 matmuls, lhsT = block-diag bf16 codebook [128, 32]
       (12 live cols padded to 32), halves writing PSUM partitions
       {0..31} / {32..63}; ACT evict (bf16) -> SBUF; DMA out 1KB per
       partition over 64 partitions.

Numerics: scores match the fp32 affine form to ~1e-7, so only pixels
with near-tied top-2 distances can flip to the other color (a handful
of 1M); the gather codebook is rounded to bf16 (rel err ~1e-3 vs the
2e-2 budget) and the output is returned as bf16 codebook colors
(lossless for single-hot pixels).  Measured rel-l2 error vs the fp32
reference on hardware: 2.55e-3.
"""

import numpy as np

H = 1024
W = 1024
K = 32
EPS = 1e-6
NCORES = 8
ROWS = H // NCORES            # 128 rows per core
NPX = ROWS * W                # 131072 pixels per core
TILE_PX = 2048                # pixels per tile (4 slots x 512)
NSLOT = 4
SLOT_N = 512                  # columns per slot
NT = NPX // TILE_PX           # 64 tiles

# bf16 term pairs (x_term_i, w_term_j) stacked along the matmul contract
# dim; i+j<=2 keeps every product term down to ~2^-26 relative.
PAIRS = [(0, 0), (0, 1), (1, 0), (1, 1), (0, 2), (2, 0)]
NBIAS = 3                     # bias bf16 terms (ones rows)
NSTK = 12 * len(PAIRS) + NBIAS  # 75 contract rows


def _build_program(n_tiles, reps=1):
    import concourse.bass as bass
    import concourse.bacc as bacc
    import concourse.tile as tile
    from concourse import mybir

    f32 = mybir.dt.float32
    bf16 = mybir.dt.bfloat16

    nc = bacc.Bacc(None, target_bir_lowering=False)
    # x rows: the stacked bf16 image terms (see PAIRS) + 3 ones rows.
    # col 512t+n <-> pixel 2048t + 512q + n (4 pixels per column, one per
    # q slot, channel rows 4c+q within each 12-row pair block).
    L = SLOT_N * n_tiles
    x = nc.dram_tensor("x", [NSTK, L], bf16, kind="ExternalInput")
    wstk = nc.dram_tensor("wstk", [NSTK, 128], bf16, kind="ExternalInput")
    # cbt cols [0:32]: gather codebook, block-diag (padded to 32 output
    # partitions so the whole evict region is written):
    # cbt[32q+k, 4c+q] = cb[k, c].  cols [32:160]: bf16 identity.
    cbt = nc.dram_tensor("cbt", [128, 160], bf16, kind="ExternalInput")
    # y[32h + 4c+q, 512s + n] = channel c of pixel col 1024s + 512h + n
    # (bf16 is lossless here: single-hot colors are exactly the bf16
    # codebook entries)
    y = nc.dram_tensor("y", [64, L // 2], bf16, kind="ExternalOutput")

    assert n_tiles % 2 == 0
    n_super = n_tiles // 2
    SUP = 2 * SLOT_N  # 1024 cols per supertile, 2 PSUM banks
    QCOL = SUP // 2   # 512 cols per gather half
    with tile.TileContext(nc) as tc:
        with (
            tc.tile_pool(name="const", bufs=1) as constp,
            tc.tile_pool(name="io", bufs=1) as iop,
            tc.tile_pool(name="work", bufs=4) as workp,
            tc.tile_pool(name="ps", bufs=3, space=bass.MemorySpace.PSUM) as psp,
            tc.tile_pool(name="pso", bufs=1, space=bass.MemorySpace.PSUM) as psop,
            tc.tile_pool(name="psq", bufs=1, space=bass.MemorySpace.PSUM) as psq,
        ):
            cbt_t = constp.tile([128, 160], bf16)
            nc.sync.dma_start(cbt_t[:], cbt[:])
            cb_t = cbt_t[:, 0:32]
            iden_t = cbt_t[:, 32:160]
            w_t = constp.tile([NSTK, 128], bf16)
            nc.sync.dma_start(w_t[:], wstk[:])

            img = iop.tile([NSTK, L], bf16, tag="img")
            nc.sync.dma_start(img[:], x[:])

            def _front(s):
                # transposed scores: 8 blocks of [128 px, (q,k)]
                ps_T = psp.tile([128, SUP], f32, tag="ps_T")
                for b in range(8):
                    col = SUP * s + 128 * b
                    nc.tensor.matmul(
                        ps_T[:, 128 * b : 128 * (b + 1)],
                        img[:, col : col + 128],
                        w_t,
                    )

                # per-pixel min over the 32 scores (DVE, straight from PSUM)
                m = workp.tile([128, 32], f32, tag="m")
                nc.vector.tensor_reduce(
                    m[:],
                    ps_T[:].rearrange("p (s k) -> p s k", k=K),
                    axis=mybir.AxisListType.X,
                    op=mybir.AluOpType.min,
                )
                return ps_T, m

            def _back(s, ps_T, m):
                # one-hot in transposed layout (DVE; m broadcast along k
                # via a zero-stride AP).  GPSIMD cannot run TensorTensor or
                # touch PSUM on this hardware, so DVE owns both passes.
                # Emitted one super late so the DVE's in-order queue has
                # min(s+1) between min(s) and is_equal(s), hiding the m
                # semaphore hop.
                onehot = workp.tile([128, SUP], bf16, tag="onehot")
                nc.vector.tensor_tensor(
                    onehot[:].rearrange("p (s k) -> p s k", k=K),
                    ps_T[:].rearrange("p (s k) -> p s k", k=K),
                    m[:].to_broadcast((128, 32, K)),
                    op=mybir.AluOpType.is_equal,
                )

                # transpose back to [(q,k), px] per block on the PE, then
                # one ACT evict (XBAR DMA transposes measured ~10x slower
                # on real hw than the sim model)
                ps_O = psop.tile([128, SUP], bf16, tag="ps_O")
                for b in range(8):
                    nc.tensor.transpose(
                        ps_O[:, 128 * b : 128 * (b + 1)],
                        onehot[:, 128 * b : 128 * (b + 1)],
                        iden_t,
                    )
                oh_sb = workp.tile([128, SUP], bf16, tag="oh_sb")
                nc.scalar.activation(
                    oh_sb[:], ps_O[:], mybir.ActivationFunctionType.Copy
                )

                # gather colors: half h -> PSUM partitions 32h..32h+11
                ps_o = psq.tile([64, QCOL], f32, tag="ps_o")
                for g in range(2):
                    nc.tensor.matmul(
                        ps_o[32 * g : 32 * g + 32, :],
                        cb_t,
                        oh_sb[:, QCOL * g : QCOL * (g + 1)],
                    )
                # colors evict on ACT (DVE is the bottleneck engine)
                o_sb = workp.tile([64, QCOL], bf16, tag="o_sb")
                nc.scalar.activation(
                    o_sb[:], ps_o[:], mybir.ActivationFunctionType.Copy
                )
                nc.sync.dma_start(y[:, QCOL * s : QCOL * (s + 1)], o_sb[:])

            def _body():
                # software-pipelined emission: is_equal and the back half
                # of super s are emitted after min(s+1)
                pend = None
                for s in range(n_super):
                    cur = (s,) + _front(s)
                    if pend is not None:
                        _back(*pend)
                    pend = cur
                _back(*pend)

            if reps == 1:
                _body()
            else:
                # hardware loop: used only for timing (program size stays
                # constant while the iteration count varies).  Two bodies
                # per For_i iteration amortize loop-control overhead.
                assert reps % 2 == 0
                with tc.For_i(0, reps // 2, 1):
                    _body()
                    _body()
    nc.compile()
    return nc


def _bf16_terms(a, n):
    """Split a float array into n bf16 terms (greedy residual)."""
    import ml_dtypes

    terms = []
    r = np.asarray(a, np.float64)
    for _ in range(n):
        t = r.astype(np.float32).astype(ml_dtypes.bfloat16)
        terms.append(t)
        r = r - t.astype(np.float64)
    return terms


def _host_consts(printability_array):
    """Pack kernel constants: wstk [75,128] bf16 and cbt [128,12] bf16."""
    import ml_dtypes

    cb = np.asarray(printability_array, np.float64).reshape(K, 3)
    w = 2.0 * (EPS - cb)                       # [K, 3]
    b = np.sum((EPS - cb) ** 2, axis=1)        # [K]
    wt = _bf16_terms(w, 3)
    bt = _bf16_terms(b, NBIAS)

    wstk = np.zeros((NSTK, 128), ml_dtypes.bfloat16)
    for p, (_, j) in enumerate(PAIRS):
        for q in range(NSLOT):
            for c in range(3):
                for k in range(K):
                    wstk[12 * p + 4 * c + q, 32 * q + k] = wt[j][k, c]
    for j in range(NBIAS):
        for q in range(NSLOT):
            wstk[12 * len(PAIRS) + j, 32 * q : 32 * q + K] = bt[j]

    cbt = np.zeros((128, 160), ml_dtypes.bfloat16)
    cbf = cb.astype(np.float32).astype(ml_dtypes.bfloat16)
    for q in range(NSLOT):
        for k in range(K):
            for c in range(3):
                cbt[32 * q + k, 4 * c + q] = cbf[k, c]
    cbt[:, 32:160] = np.eye(128, dtype=ml_dtypes.bfloat16)
    return wstk, cbt


_PROG_CACHE = {}


def _pack_x(flat3):
    """[3, npx] fp32 -> [75, npx/4] bf16 stacked-term image."""
    import ml_dtypes

    npx = flat3.shape[1]
    nt = npx // TILE_PX
    ncols = nt * SLOT_N
    v = flat3.reshape(3, nt, NSLOT, SLOT_N)          # (c, t, q, n)
    v12 = v.transpose(0, 2, 1, 3).reshape(12, ncols)  # rows 4c+q
    xt = _bf16_terms(v12, 3)
    out = np.empty((NSTK, ncols), ml_dtypes.bfloat16)
    for p, (i, _) in enumerate(PAIRS):
        out[12 * p : 12 * (p + 1)] = xt[i]
    out[12 * len(PAIRS) :] = 1.0
    return out


def _unpack_y(y128):
    """[64, ncols/2] -> [3, npx] inverse of the device output layout."""
    ncols2 = y128.shape[1]
    n_super = ncols2 // 512
    # y128[32h + j, 512s + n] = v12[j, 1024s + 512h + n], j = 4c+q
    yr = np.asarray(y128).astype(np.float32).reshape(2, 32, n_super, 512)[:, :12]
    v12 = yr.transpose(1, 2, 0, 3).reshape(12, n_super * 1024)
    nt = (ncols2 * 2) // SLOT_N
    v = v12.reshape(3, NSLOT, nt, SLOT_N)            # (c, q, t, n)
    return v.transpose(0, 2, 1, 3).reshape(3, nt * TILE_PX)


def kernel(adv_patch, printability_array):
    from concourse.bass_utils import run_bass_kernel_spmd

    adv_patch = np.ascontiguousarray(adv_patch, dtype=np.float32)
    wstk, cbt = _host_consts(printability_array)

    if NT not in _PROG_CACHE:
        _PROG_CACHE[NT] = _build_program(NT)
    nc = _PROG_CACHE[NT]

    in_maps = []
    for i in range(NCORES):
        xs = adv_patch[:, i * ROWS : (i + 1) * ROWS, :].reshape(3, NPX)
        in_maps.append({"x": _pack_x(xs), "wstk": wstk, "cbt": cbt})

    res = run_bass_kernel_spmd(nc, in_maps, list(range(NCORES)))

    out = np.empty((1, 3, H, W), np.float32)
    for i in range(NCORES):
        out[0, :, i * ROWS : (i + 1) * ROWS, :] = _unpack_y(
            res.results[i]["y"]
        ).reshape(3, ROWS, W)
    return out


# revision 22
# speedup vs baseline: 1.5312x; 1.1011x over previous
"""DifColorQuantization Trainium2 kernel.

Math (per pixel p, codebook color k):
    ref:  argmin_k sqrt(sum_c (x_c - cb_kc + eps)^2 + eps) ; out = cb[argmin]
    sqrt/+eps are monotone, so rank by the k-dependent part of the expanded
    square:  s_k = sum_c w_kc * x_c + b_k,  w_kc = 2*(eps-cb_kc),
    b_k = sum_c (eps-cb_kc)^2  (the sum_c x_c^2 term is k-independent).

Device pipeline per core (H sharded 8 ways, 131072 px/core, 32 supertiles
of 4096 px = 1024 cols x 4 q-slots; block b = 128 pixel columns):
    1. PE scores, transposed layout: per block, ONE bf16 matmul with
       lhsT = image chunk [75, 128] (stationary), rhs = stacked weights
       [75, 128] -> PSUM [128 px, (q,k)].  The 75 contract rows hold the
       fp32 x fp32 products expanded into bf16-term pairs (x split into 3
       bf16 terms, w into 3; all pairs with i+j<=2 stacked along the
       contract dim, + 3 ones rows for the 3 bias terms).  PE matmul cost
       depends only on the 128 moving columns, so this runs 4x faster
       than an fp32 matmul at ~fp32 accuracy (score err ~1e-7).
    2. DVE reduce min over k segments -> m [128, 32]
    3. DVE tensor_tensor is_equal(scores_T, m broadcast via a zero-stride
       AP) -> one-hot_T bf16 in SBUF.  (Pool/GPSIMD cannot run
       TensorTensor nor read PSUM on this hardware, so DVE owns both the
       min and the compare; DVE is the bottleneck engine.)
    4. PE transpose per 128-block -> one-hot [(q,k), px] in PSUM; one ACT
       evict -> SBUF.  (XBAR DMA transposes measured ~10us/supertile on
       real hw vs the 0.9us sim model - do not use them here.)
    5. PE gather: 2 matmuls, lhsT = block-diag bf16 codebook [128, 32]
       (12 live cols padded to 32), halves writing PSUM partitions
       {0..31} / {32..63}; ACT evict (bf16) -> SBUF; DMA out 1KB per
       partition over 64 partitions.

Numerics: scores match the fp32 affine form to ~1e-7, so only pixels
with near-tied top-2 distances can flip to the other color (a handful
of 1M); the gather codebook is rounded to bf16 (rel err ~1e-3 vs the
2e-2 budget) and the output is returned as bf16 codebook colors
(lossless for single-hot pixels).  Measured rel-l2 error vs the fp32
reference on hardware: 2.55e-3.
"""

import numpy as np

H = 1024
W = 1024
K = 32
EPS = 1e-6
NCORES = 8
ROWS = H // NCORES            # 128 rows per core
NPX = ROWS * W                # 131072 pixels per core
TILE_PX = 2048                # pixels per tile (4 slots x 512)
NSLOT = 4
SLOT_N = 512                  # columns per slot
NT = NPX // TILE_PX           # 64 tiles

# bf16 term pairs (x_term_i, w_term_j) stacked along the matmul contract
# dim; i+j<=2 keeps every product term down to ~2^-26 relative.
PAIRS = [(0, 0), (0, 1), (1, 0), (1, 1), (0, 2), (2, 0)]
NBIAS = 3                     # bias bf16 terms (ones rows)
NSTK = 12 * len(PAIRS) + NBIAS  # 75 contract rows


def _build_program(n_tiles, reps=1):
    import concourse.bass as bass
    import concourse.bacc as bacc
    import concourse.tile as tile
    from concourse import mybir

    f32 = mybir.dt.float32
    bf16 = mybir.dt.bfloat16

    nc = bacc.Bacc(None, target_bir_lowering=False)
    # x rows: the stacked bf16 image terms (see PAIRS) + 3 ones rows.
    # col 512t+n <-> pixel 2048t + 512q + n (4 pixels per column, one per
    # q slot, channel rows 4c+q within each 12-row pair block).
    L = SLOT_N * n_tiles
    x = nc.dram_tensor("x", [NSTK, L], bf16, kind="ExternalInput")
    wstk = nc.dram_tensor("wstk", [NSTK, 128], bf16, kind="ExternalInput")
    # cbt cols [0:32]: gather codebook, block-diag (padded to 32 output
    # partitions so the whole evict region is written):
    # cbt[32q+k, 4c+q] = cb[k, c].  cols [32:160]: bf16 identity.
    cbt = nc.dram_tensor("cbt", [128, 160], bf16, kind="ExternalInput")
    # y[32h + 4c+q, 512s + n] = channel c of pixel col 1024s + 512h + n
    # (bf16 is lossless here: single-hot colors are exactly the bf16
    # codebook entries)
    y = nc.dram_tensor("y", [64, L // 2], bf16, kind="ExternalOutput")

    assert n_tiles % 2 == 0
    n_super = n_tiles // 2
    SUP = 2 * SLOT_N  # 1024 cols per supertile, 2 PSUM banks
    QCOL = SUP // 2   # 512 cols per gather half
    with tile.TileContext(nc) as tc:
        with (
            tc.tile_pool(name="const", bufs=1) as constp,
            tc.tile_pool(name="io", bufs=1) as iop,
            tc.tile_pool(name="work", bufs=4) as workp,
            tc.tile_pool(name="ps", bufs=3, space=bass.MemorySpace.PSUM) as psp,
            tc.tile_pool(name="pso", bufs=1, space=bass.MemorySpace.PSUM) as psop,
            tc.tile_pool(name="psq", bufs=1, space=bass.MemorySpace.PSUM) as psq,
        ):
            cbt_t = constp.tile([128, 160], bf16)
            nc.sync.dma_start(cbt_t[:], cbt[:])
            cb_t = cbt_t[:, 0:32]
            iden_t = cbt_t[:, 32:160]
            w_t = constp.tile([NSTK, 128], bf16)
            nc.sync.dma_start(w_t[:], wstk[:])

            img = iop.tile([NSTK, L], bf16, tag="img")
            nc.sync.dma_start(img[:], x[:])

            def _front(s):
                # transposed scores: 8 blocks of [128 px, (q,k)]
                ps_T = psp.tile([128, SUP], f32, tag="ps_T")
                for b in range(8):
                    col = SUP * s + 128 * b
                    nc.tensor.matmul(
                        ps_T[:, 128 * b : 128 * (b + 1)],
                        img[:, col : col + 128],
                        w_t,
                    )

                # per-pixel min over the 32 scores (DVE, straight from PSUM)
                m = workp.tile([128, 32], f32, tag="m")
                nc.vector.tensor_reduce(
                    m[:],
                    ps_T[:].rearrange("p (s k) -> p s k", k=K),
                    axis=mybir.AxisListType.X,
                    op=mybir.AluOpType.min,
                )
                return ps_T, m

            def _mid(s, ps_T, m):
                # one-hot in transposed layout (DVE; m broadcast along k
                # via a zero-stride AP).  GPSIMD cannot run TensorTensor or
                # touch PSUM on this hardware, so DVE owns both passes.
                # Emitted two supers late so the DVE's in-order queue has
                # min(s+1), min(s+2) between min(s) and is_equal(s),
                # hiding the m semaphore hop.
                onehot = workp.tile([128, SUP], bf16, tag="onehot")
                nc.vector.tensor_tensor(
                    onehot[:].rearrange("p (s k) -> p s k", k=K),
                    ps_T[:].rearrange("p (s k) -> p s k", k=K),
                    m[:].to_broadcast((128, 32, K)),
                    op=mybir.AluOpType.is_equal,
                )

                # transpose back to [(q,k), px] per block on the PE, then
                # one ACT evict (XBAR DMA transposes measured ~10x slower
                # on real hw than the sim model)
                ps_O = psop.tile([128, SUP], bf16, tag="ps_O")
                for b in range(8):
                    nc.tensor.transpose(
                        ps_O[:, 128 * b : 128 * (b + 1)],
                        onehot[:, 128 * b : 128 * (b + 1)],
                        iden_t,
                    )
                oh_sb = workp.tile([128, SUP], bf16, tag="oh_sb")
                nc.scalar.activation(
                    oh_sb[:], ps_O[:], mybir.ActivationFunctionType.Copy
                )
                return oh_sb

            def _tail(s, oh_sb):
                # gather colors: half h -> PSUM partitions 32h..32h+11.
                # Emitted one more super late so the PE's in-order queue
                # never parks on the oh_sb evict semaphore in front of the
                # next super's score matmuls.
                ps_o = psq.tile([64, QCOL], f32, tag="ps_o")
                for g in range(2):
                    nc.tensor.matmul(
                        ps_o[32 * g : 32 * g + 32, :],
                        cb_t,
                        oh_sb[:, QCOL * g : QCOL * (g + 1)],
                    )
                # colors evict on ACT (DVE is the bottleneck engine)
                o_sb = workp.tile([64, QCOL], bf16, tag="o_sb")
                nc.scalar.activation(
                    o_sb[:], ps_o[:], mybir.ActivationFunctionType.Copy
                )
                nc.sync.dma_start(y[:, QCOL * s : QCOL * (s + 1)], o_sb[:])

            def _body():
                # 3-stage software-pipelined emission: fronts (scores+min)
                # run two supers ahead of the compare/transpose stage,
                # which runs one super ahead of the gather/output tail.
                fronts = {}
                mids = {}
                for s in range(n_super + 3):
                    if s < n_super:
                        fronts[s] = _front(s)
                    if 0 <= s - 2 < n_super:
                        mids[s - 2] = _mid(s - 2, *fronts.pop(s - 2))
                    if 0 <= s - 3 < n_super:
                        _tail(s - 3, mids.pop(s - 3))

            if reps == 1:
                _body()
            else:
                # hardware loop: used only for timing (program size stays
                # constant while the iteration count varies).  Two bodies
                # per For_i iteration amortize loop-control overhead.
                assert reps % 2 == 0
                with tc.For_i(0, reps // 2, 1):
                    _body()
                    _body()
    nc.compile()
    return nc


def _bf16_terms(a, n):
    """Split a float array into n bf16 terms (greedy residual)."""
    import ml_dtypes

    terms = []
    r = np.asarray(a, np.float64)
    for _ in range(n):
        t = r.astype(np.float32).astype(ml_dtypes.bfloat16)
        terms.append(t)
        r = r - t.astype(np.float64)
    return terms


def _host_consts(printability_array):
    """Pack kernel constants: wstk [75,128] bf16 and cbt [128,12] bf16."""
    import ml_dtypes

    cb = np.asarray(printability_array, np.float64).reshape(K, 3)
    w = 2.0 * (EPS - cb)                       # [K, 3]
    b = np.sum((EPS - cb) ** 2, axis=1)        # [K]
    wt = _bf16_terms(w, 3)
    bt = _bf16_terms(b, NBIAS)

    wstk = np.zeros((NSTK, 128), ml_dtypes.bfloat16)
    for p, (_, j) in enumerate(PAIRS):
        for q in range(NSLOT):
            for c in range(3):
                for k in range(K):
                    wstk[12 * p + 4 * c + q, 32 * q + k] = wt[j][k, c]
    for j in range(NBIAS):
        for q in range(NSLOT):
            wstk[12 * len(PAIRS) + j, 32 * q : 32 * q + K] = bt[j]

    cbt = np.zeros((128, 160), ml_dtypes.bfloat16)
    cbf = cb.astype(np.float32).astype(ml_dtypes.bfloat16)
    for q in range(NSLOT):
        for k in range(K):
            for c in range(3):
                cbt[32 * q + k, 4 * c + q] = cbf[k, c]
    cbt[:, 32:160] = np.eye(128, dtype=ml_dtypes.bfloat16)
    return wstk, cbt


_PROG_CACHE = {}


def _pack_x(flat3):
    """[3, npx] fp32 -> [75, npx/4] bf16 stacked-term image."""
    import ml_dtypes

    npx = flat3.shape[1]
    nt = npx // TILE_PX
    ncols = nt * SLOT_N
    v = flat3.reshape(3, nt, NSLOT, SLOT_N)          # (c, t, q, n)
    v12 = v.transpose(0, 2, 1, 3).reshape(12, ncols)  # rows 4c+q
    xt = _bf16_terms(v12, 3)
    out = np.empty((NSTK, ncols), ml_dtypes.bfloat16)
    for p, (i, _) in enumerate(PAIRS):
        out[12 * p : 12 * (p + 1)] = xt[i]
    out[12 * len(PAIRS) :] = 1.0
    return out


def _unpack_y(y128):
    """[64, ncols/2] -> [3, npx] inverse of the device output layout."""
    ncols2 = y128.shape[1]
    n_super = ncols2 // 512
    # y128[32h + j, 512s + n] = v12[j, 1024s + 512h + n], j = 4c+q
    yr = np.asarray(y128).astype(np.float32).reshape(2, 32, n_super, 512)[:, :12]
    v12 = yr.transpose(1, 2, 0, 3).reshape(12, n_super * 1024)
    nt = (ncols2 * 2) // SLOT_N
    v = v12.reshape(3, NSLOT, nt, SLOT_N)            # (c, q, t, n)
    return v.transpose(0, 2, 1, 3).reshape(3, nt * TILE_PX)


def kernel(adv_patch, printability_array):
    from concourse.bass_utils import run_bass_kernel_spmd

    adv_patch = np.ascontiguousarray(adv_patch, dtype=np.float32)
    wstk, cbt = _host_consts(printability_array)

    if NT not in _PROG_CACHE:
        _PROG_CACHE[NT] = _build_program(NT)
    nc = _PROG_CACHE[NT]

    in_maps = []
    for i in range(NCORES):
        xs = adv_patch[:, i * ROWS : (i + 1) * ROWS, :].reshape(3, NPX)
        in_maps.append({"x": _pack_x(xs), "wstk": wstk, "cbt": cbt})

    res = run_bass_kernel_spmd(nc, in_maps, list(range(NCORES)))

    out = np.empty((1, 3, H, W), np.float32)
    for i in range(NCORES):
        out[0, :, i * ROWS : (i + 1) * ROWS, :] = _unpack_y(
            res.results[i]["y"]
        ).reshape(3, ROWS, W)
    return out
